# revision 39
# baseline (speedup 1.0000x reference)
"""Trainium2 Bass kernel for nn_DressedQuantumCircuit.

The 4-qubit dressed quantum circuit in the reference collapses to a
closed form.  With theta_q = (pi/2) * tanh(x_q) and w = q_params:

    out[:, 0] = -sin(w0) * (1/2)     * cos(theta_1 + pi/4)
    out[:, 1] = -sin(w1) * (sqrt2/2) * cos(theta_3 + pi/4)
    out[:, 2] = -sin(w2) * (sqrt2/2) * cos(theta_0)
    out[:, 3] = -sin(w3) * (1/2)     * cos(theta_2 + pi/4)

(derivation: the H + RZ + CRZ layers produce a uniform-magnitude state
with diagonal phases; SWAPs permute wires; RY(w) conjugates Z into
cos(w)Z - sin(w)X; <Z> = 0 and <X_q> reduces to the cosines above.)

Device kernel: pure elementwise map over [B, 4] f32 — Tanh (ACT), Sin
with affine prescale (ACT), per-column coefficient multiply (DVE).
The HW Sin spline is accurate only for |u| <= pi, so the cosines are
phrased to keep arguments inside (-3pi/4, pi):
    cols 0,1,3:  cos(t + pi/4) = -sin(t - pi/4)
    col  2:      cos(t)        =  sin(t + pi/2)
Pure data parallel over the batch: each of 8 cores does B/8 rows.
"""

import math

import numpy as np

import concourse.bacc as bacc
import concourse.bass as bass
import concourse.mybir as mybir
from contextlib import ExitStack
from concourse.bass_utils import run_bass_kernel_spmd
from concourse.hw_specs import get_activation_tables

N_CORES = 8
BATCH = 524288
NQ = 4
B_LOCAL = BATCH // N_CORES          # 65536 rows per core
P = 128                             # SBUF partitions
FREE = B_LOCAL * NQ // P            # 2048 f32 per partition
NCHUNK = 2                          # pipeline chunks per core
# uneven split: chunk0 a bit larger so the output queue opens earlier
# while chunk1's compute+store tail shrinks (end-to-end model optimum
# sits near 0.56-0.6, not 0.5)
CS = (1152, 896)                    # f32 per partition per chunk
COFF = (0, 1152)
assert sum(CS) == FREE

# out column j reads input column PERM[j] = (1, 3, 0, 2)
SIN_BIAS = (-0.25 * math.pi, -0.25 * math.pi, 0.5 * math.pi, -0.25 * math.pi)
# static output coefficients (times -sin(w_j) at runtime); the -sin
# identity sign for cols 0,1,3 is folded in
COEF = (-0.5, -math.sqrt(2.0) / 2.0, math.sqrt(2.0) / 2.0, -0.5)

TRACE = False          # set by test.py to capture an NTFF profile
LAST_RESULT = None     # BassKernelResults of the last run when TRACE

_cached_nc = None


def _build():
    global _cached_nc
    if _cached_nc is not None:
        return _cached_nc

    nc = bacc.Bacc(trn_type="TRN2")
    x = nc.declare_dram_parameter("x", [B_LOCAL, NQ], mybir.dt.float32, isOutput=False)
    # per-partition constants: cols 0-3 = output coefs A_j, cols 4-7 = sin biases
    acoef = nc.declare_dram_parameter(
        "acoef", [P, 2 * NQ], mybir.dt.float32, isOutput=False
    )
    y = nc.declare_dram_parameter("y", [B_LOCAL, NQ], mybir.dt.float32, isOutput=True)

    # flat views: partition p holds 512 consecutive rows (x4 cols, interleaved)
    xv = x.rearrange("(p n) f -> p (n f)", p=P)   # [128, 2048]
    yv = y.rearrange("(p n) f -> p (n f)", p=P)

    AF = mybir.ActivationFunctionType
    HALF_PI = 0.5 * math.pi

    # one act table set that covers BOTH Tanh and Sin, so the kernel pays a
    # single ACT_TABLE_LOAD (overlapped with the input DMA) instead of the
    # per-function alternation the auto-inserter would produce
    tables = get_activation_tables(nc.m.arch)
    both_idx = next(
        (
            i
            for i, fns in enumerate(tables.values())
            if {AF.Tanh, AF.Sin} <= set(fns)
        ),
        None,
    )

    # Raw bass (no Tile): the kernel is ~30 instructions, and hand-rolled
    # semaphores avoid the Tile entry sems (~1us) + exit drain/barrier
    # cascade (~2.4us) that dominate a kernel this small.
    with ExitStack() as ctx:
        sbuf = lambda name, shape: ctx.enter_context(
            nc.sbuf_tensor(name, shape, mybir.dt.float32)
        )
        at = sbuf("at", [P, 2 * NQ])
        xts = [sbuf(f"xt{i}", [P, CS[i]]) for i in range(NCHUNK)]
        tts = [sbuf(f"tt{i}", [P, CS[i]]) for i in range(NCHUNK)]
        yts = [sbuf(f"yt{i}", [P, CS[i]]) for i in range(NCHUNK)]
        ots = [sbuf(f"ot{i}", [P, CS[i]]) for i in range(NCHUNK)]

        s_x = ctx.enter_context(nc.semaphore("s_x"))
        s_at = ctx.enter_context(nc.semaphore("s_at"))
        s_sin = ctx.enter_context(nc.semaphore("s_sin"))
        s_mul = ctx.enter_context(nc.semaphore("s_mul"))
        s_y = ctx.enter_context(nc.semaphore("s_y"))

        block = ctx.enter_context(nc.Block())

        @block.sync
        def _(sync):
            # input chunks strictly serialized: concurrent DMAs interleave
            # at packet granularity (chunk0 would then only complete near
            # the end of the whole stream); serializing gives chunk0 the
            # full bandwidth so compute starts ~1.3us earlier
            for i in range(NCHUNK):
                if i > 0:
                    # 15/16 engine-increments of the previous chunk: the
                    # engines finish within a tight window, so this only
                    # hides part of the completion-receipt latency
                    sync.wait_ge(s_x, 16 * i - 1)
                sync.dma_start(
                    xts[i][:], xv[:, COFF[i] : COFF[i] + CS[i]]
                ).then_inc(s_x, 16)
            for i in range(NCHUNK):
                sync.wait_ge(s_mul, 3 * (i + 1))
                sync.dma_start(
                    yv[:, COFF[i] : COFF[i] + CS[i]], ots[i][:]
                ).then_inc(s_y, 16)
            sync.wait_ge(s_y, 16 * NCHUNK)

        @block.scalar
        def _(scalar):
            # table set covering BOTH Tanh and Sin: one load, overlapping
            # the input DMA, instead of per-function alternation (if no such
            # set exists, the bacc auto-inserter still keeps it correct)
            if both_idx is not None:
                load = mybir.InstLoadActFuncSet(
                    name=nc.get_next_instruction_name(), ins=[], outs=[]
                )
                scalar.add_instruction(load)
                load.act_func_set_id = both_idx
                load.engine = mybir.EngineType.Activation
            # coef load on the ACT HWDGE queue; its descriptor-gen overlaps
            # the table load on the ACT datapath
            scalar.dma_start(at[:], acoef[:]).then_inc(s_at, 16)
            scalar.wait_ge(s_at, 16)
            for i in range(NCHUNK):
                scalar.wait_ge(s_x, 16 * (i + 1))
                scalar.activation(tts[i][:], xts[i][:], AF.Tanh)
                tt3 = tts[i].rearrange("p (n f) -> p n f", f=NQ)
                yt3 = yts[i].rearrange("p (n f) -> p n f", f=NQ)
                # cols 0,1 <- sin((pi/2) t_{1,3} - pi/4): strided pair
                scalar.activation(
                    yt3[:, :, 0:2], tt3[:, :, 1::2], AF.Sin,
                    bias=at[:, NQ : NQ + 1], scale=HALF_PI,
                ).then_inc(s_sin, 1)
                # col 2 <- sin((pi/2) t_0 + pi/2)
                scalar.activation(
                    yt3[:, :, 2], tt3[:, :, 0], AF.Sin,
                    bias=at[:, NQ + 2 : NQ + 3], scale=HALF_PI,
                ).then_inc(s_sin, 1)
                # col 3 <- sin((pi/2) t_2 - pi/4)
                scalar.activation(
                    yt3[:, :, 3], tt3[:, :, 2], AF.Sin,
                    bias=at[:, NQ + 3 : NQ + 4], scale=HALF_PI,
                ).then_inc(s_sin, 1)

        @block.vector
        def _(vector):
            vector.wait_ge(s_at, 16)
            for i in range(NCHUNK):
                yt3 = yts[i].rearrange("p (n f) -> p n f", f=NQ)
                ot3 = ots[i].rearrange("p (n f) -> p n f", f=NQ)
                npr = CS[i] // NQ
                a01 = (
                    at[:, 0:2]
                    .rearrange("p (n f) -> p n f", n=1)
                    .to_broadcast((P, npr, 2))
                )
                vector.wait_ge(s_sin, 3 * i + 1)
                vector.tensor_mul(ot3[:, :, 0:2], yt3[:, :, 0:2], a01).then_inc(
                    s_mul, 1
                )
                vector.wait_ge(s_sin, 3 * i + 2)
                vector.tensor_scalar_mul(
                    ot3[:, :, 2], yt3[:, :, 2], at[:, 2:3]
                ).then_inc(s_mul, 1)
                vector.wait_ge(s_sin, 3 * i + 3)
                vector.tensor_scalar_mul(
                    ot3[:, :, 3], yt3[:, :, 3], at[:, 3:4]
                ).then_inc(s_mul, 1)

    nc.finalize()  # Bacc: runs compile() incl. the 1-wait-per-inst split
    _cached_nc = nc
    return nc


def kernel(input_features: np.ndarray, q_params: np.ndarray) -> np.ndarray:
    global LAST_RESULT
    x = np.ascontiguousarray(np.asarray(input_features, dtype=np.float32))
    w = np.asarray(q_params, dtype=np.float64).reshape(NQ)
    assert x.shape == (BATCH, NQ), x.shape

    # runtime output coefficients + sin biases, replicated across partitions
    a = -np.sin(w) * np.array(COEF, dtype=np.float64)
    row = np.concatenate([a, np.array(SIN_BIAS, dtype=np.float64)])
    a_rep = np.ascontiguousarray(np.tile(row[None, :], (P, 1)).astype(np.float32))

    nc = _build()
    shards = x.reshape(N_CORES, B_LOCAL, NQ)
    in_maps = [{"x": shards[i], "acoef": a_rep} for i in range(N_CORES)]

    res = run_bass_kernel_spmd(nc, in_maps, list(range(N_CORES)), trace=TRACE)
    if TRACE:
        LAST_RESULT = res

    out = np.concatenate([res.results[i]["y"] for i in range(N_CORES)], axis=0)
    return out.astype(np.float32, copy=False)
